# revision 7
# baseline (speedup 1.0000x reference)
"""MoE expert-parallel FFN kernel for TRN2 (8 NeuronCores).

Reference computation (per expert e):
    h = gelu(x_e @ W1[e] + b1[e]);  y_e = h @ W2[e] + b2[e]
with x = inputs[0].reshape(E, CAP, D), E=8, CAP=4096, D=1024, F=4096.

Sharding: expert parallel - core e owns expert e and its CAP-token slice.
No cross-core communication.

Per-core dataflow (bf16 operands = full PE rate + FWL fast weight load):
  mm1: hT[f, tok] = W1[d, f].T @ xT[d, tok]   (K=D, lhsT=W1 tile [d,f])
       fused bias+gelu on psum eviction (ACT, per-partition bias = b1)
  mm2: yT[d, tok] = W2[f, d].T @ hT[f, tok]   (K=F, lhsT=W2 tile [f,d])
       b2 added via ACT bias on psum eviction; output stays transposed
       [D, CAP] in DRAM and the host transposes on gather.
Both matmuls keep one stationary [128,128] weight tile live for 4
consecutive N=512 matmuls (TC=2048 token chunks), so with walrus
redundant-LDW elision only 1 weight load is paid per 4 matmuls.
"""

import sys

if "/opt/trn_rl_repo" not in sys.path:
    sys.path.insert(0, "/opt/trn_rl_repo")

from contextlib import ExitStack

import numpy as np

import concourse.bacc as bacc
import concourse.tile as tile
from concourse import mybir
from concourse.bass_utils import run_bass_kernel_spmd

E, CAP, D, F = 8, 4096, 1024, 4096
P = 128
TC = 2048            # tokens per chunk
NT = CAP // TC       # token chunks per core (2)
KD = D // P          # k-tiles for mm1 (8)
FM = F // P          # f chunks (32)
NDT = D // P         # output d tiles for mm2 (8)
NQ = TC // 512       # 512-wide token quarters per chunk (4)

F32 = mybir.dt.float32
F32R = mybir.dt.float32r
BF16 = mybir.dt.bfloat16
GELU = mybir.ActivationFunctionType.Gelu_apprx_tanh
IDENT = mybir.ActivationFunctionType.Identity

MM_DT = BF16   # matmul operand dtype
LDW_OPT = False  # flip walrus --enable-ldw-opt (redundant weight-load elision)

_cache = {}


def _install_ldw_opt_patch():
    """Flip walrus's --enable-ldw-opt flag (redundant weight-load elision).

    Consecutive matmuls in this kernel share stationary operands; eliding
    the redundant LDWEIGHTS removes dead weight-load time on the PE.
    (Known-broken for fp32/fp32r weights; bf16 is the supported path.)
    """
    import concourse.bass_utils as bu

    if getattr(bu, "_ldw_opt_patched", False):
        return
    orig = bu.run_command

    def patched(cmd, **kw):
        if LDW_OPT and isinstance(cmd, list):
            cmd = ["--enable-ldw-opt=true" if c == "--enable-ldw-opt=false" else c
                   for c in cmd]
        return orig(cmd, **kw)

    bu.run_command = patched
    bu._ldw_opt_patched = True


_install_ldw_opt_patch()


def _ldw_sig(inst):
    a = inst.ins[0]
    return (
        a.memref.name if hasattr(a.memref, "name") else str(a.memref),
        a.offset,
        str(a.ap),
        str(a.dtype),
        str(inst.tile_position),
        str(inst.tile_size),
        str(inst.perf_mode),
        str(inst.is_transpose),
    )


def _elide_redundant_ldweights(nc):
    """Drop back-to-back Ldweights that reload the identical stationary tile.

    bass lowers every matmul to an explicit Ldweights + non-self-loading
    Matmult pair; consecutive matmuls here share the stationary operand 4x,
    so 3/4 of the weight loads re-read identical SBUF data into the PE
    array. Eliding them is safe when the Ldweights carries no semaphore
    waits/updates (an SBUF refill of the weight tile always forces a wait
    onto the next Ldweights) and only plain Matmults ran since the load.
    """
    removed = 0
    for fn in nc.m.functions:
        for blk in fn.blocks:
            insts = blk.instructions
            keep = []
            last = None
            for inst in insts:
                tname = type(inst).__name__
                if tname == "InstLdweights":
                    sig = _ldw_sig(inst)
                    if sig == last and not inst.has_wait() and not inst.has_update():
                        removed += 1
                        continue
                    last = sig
                elif tname == "InstMatmult":
                    if getattr(inst, "is_transpose", False):
                        last = None
                elif str(getattr(inst, "engine", "")) .endswith("PE"):
                    last = None  # unknown PE instruction: array state unclear
                keep.append(inst)
            if removed:
                insts[:] = keep
    return removed


def _build(mm_dt=None, repeat=1):
    if mm_dt is None:
        mm_dt = MM_DT
    nc = bacc.Bacc("TRN2", target_bir_lowering=False, debug=False)

    xt = nc.dram_tensor("xt", [NT, P, KD, TC], mm_dt, kind="ExternalInput")
    w1t = nc.dram_tensor("w1t", [FM, P, KD, P], mm_dt, kind="ExternalInput")
    w2t = nc.dram_tensor("w2t", [NDT, P, FM, P], mm_dt, kind="ExternalInput")
    b1t = nc.dram_tensor("b1t", [P, FM], F32, kind="ExternalInput")
    b2t = nc.dram_tensor("b2t", [P, NDT], F32, kind="ExternalInput")
    y = nc.dram_tensor("y", [D, CAP], F32, kind="ExternalOutput")  # yT layout

    with tile.TileContext(nc) as tc:
        with ExitStack() as ctx:
            const = ctx.enter_context(tc.tile_pool(name="const", bufs=1))
            xpool = ctx.enter_context(tc.tile_pool(name="x", bufs=1))
            htpool = ctx.enter_context(tc.tile_pool(name="ht", bufs=1))
            w1pool = ctx.enter_context(tc.tile_pool(name="w1", bufs=3))
            w2pool = ctx.enter_context(tc.tile_pool(name="w2", bufs=3))
            ypool = ctx.enter_context(tc.tile_pool(name="yev", bufs=4))
            psum = ctx.enter_context(tc.tile_pool(name="psum", bufs=8, space="PSUM"))

            b1_sb = const.tile([P, FM], F32, name=f"b1_sb_ldw{int(LDW_OPT)}")
            nc.sync.dma_start(b1_sb[:], b1t.ap())
            b2_sb = const.tile([P, NDT], F32)
            nc.sync.dma_start(b2_sb[:], b2t.ap())

            xt_r = xt.ap()    # [NT, P, KD, TC]
            w1_r = w1t.ap()   # [FM, P, KD, P]
            w2_r = w2t.ap()   # [NDT, P, FM, P]
            y_r = y.ap()      # [D, CAP]

            for t in [t for _ in range(repeat) for t in range(NT)]:
                # weight tiles for fm=0,1 issued ahead of the x chunk DMAs so
                # the first matmul groups aren't queued behind all of x
                w1_tiles = [
                    w1pool.tile([P, KD, P], mm_dt, tag="w1", name="w1p")
                    for _ in range(2)
                ]
                for i in range(2):
                    nc.sync.dma_start(w1_tiles[i][:], w1_r[i])

                x_sb = xpool.tile([P, KD, TC], mm_dt, tag="x")
                for k in range(KD):
                    nc.sync.dma_start(x_sb[:, k], xt_r[t, :, k])

                # prefetch mm2's first weight tile during the mm1 phase
                w2_next = w2pool.tile([P, FM, P], mm_dt, tag="w2", name="w2p")
                nc.sync.dma_start(w2_next[:], w2_r[0])

                ht_sb = htpool.tile([P, FM, TC], mm_dt, tag="ht")

                # --- mm1: hT[f, tok] += W1.T @ xT, fused bias+gelu ---
                # k-outer with all 4 token-quarter psums live: the 4
                # consecutive matmuls share the stationary w1[:, k] slice;
                # w1 streams 2 tiles ahead of consumption
                for fm in range(FM):
                    w1_sb = w1_tiles[fm % 2]
                    if fm + 2 < FM:
                        w1_tiles[fm % 2] = w1pool.tile([P, KD, P], mm_dt,
                                                       tag="w1", name="w1p")
                        nc.sync.dma_start(w1_tiles[fm % 2][:], w1_r[fm + 2])
                    ps_h = [
                        psum.tile([P, 512], F32, tag="ps", name="psh")
                        for _ in range(NQ)
                    ]
                    for k in range(KD):
                        for q in range(NQ):
                            nc.tensor.matmul(
                                ps_h[q][:],
                                w1_sb[:, k],
                                x_sb[:, k, q * 512:(q + 1) * 512],
                                start=(k == 0),
                                stop=(k == KD - 1),
                            )
                    for q in range(NQ):
                        nc.scalar.activation(
                            ht_sb[:, fm, q * 512:(q + 1) * 512],
                            ps_h[q][:],
                            GELU,
                            bias=b1_sb[:, fm:fm + 1],
                        )

                # --- mm2: yT[d, tok] += W2.T @ hT, b2 added on eviction ---
                for dt in range(NDT):
                    w2_sb = w2_next
                    if dt + 1 < NDT:
                        w2_next = w2pool.tile([P, FM, P], mm_dt, tag="w2",
                                              name="w2p")
                        nc.sync.dma_start(w2_next[:], w2_r[dt + 1])
                    ps_y = [
                        psum.tile([P, 512], F32, tag="ps", name="psy")
                        for _ in range(NQ)
                    ]
                    for fm in range(FM):
                        for q in range(NQ):
                            nc.tensor.matmul(
                                ps_y[q][:],
                                w2_sb[:, fm],
                                ht_sb[:, fm, q * 512:(q + 1) * 512],
                                start=(fm == 0),
                                stop=(fm == FM - 1),
                            )
                    for q in range(NQ):
                        y_sb = ypool.tile([P, 512], F32, tag="y")
                        nc.scalar.activation(
                            y_sb[:], ps_y[q][:], IDENT, bias=b2_sb[:, dt:dt + 1]
                        )
                        nc.sync.dma_start(
                            y_r[dt * P:(dt + 1) * P,
                                t * TC + q * 512:t * TC + (q + 1) * 512],
                            y_sb[:],
                        )

    nc.compile()
    n = _elide_redundant_ldweights(nc)
    expect = 3 * repeat * NT * (FM * KD + NDT * FM)
    assert n >= expect - 16 * repeat, (n, expect)
    return nc


def _wire_np_dtype(mm_dt):
    if mm_dt == BF16:
        import ml_dtypes

        return ml_dtypes.bfloat16
    return np.float32


def _prep_core_inputs(inputs, W1, b1, W2, b2, e, wdt):
    x_e = inputs[0, e * CAP:(e + 1) * CAP, :]          # [CAP, D]
    # xt[t][p][k][c] == x_e[t*TC + c, k*P + p]
    xt = np.ascontiguousarray(
        x_e.T.reshape(KD, P, NT, TC).transpose(2, 1, 0, 3)
    ).astype(wdt)
    # w1t[fm][p][k][f] == W1[k*P + p, fm*P + f]
    w1t = np.ascontiguousarray(
        W1[e].reshape(KD, P, FM, P).transpose(2, 1, 0, 3)
    ).astype(wdt)
    # w2t[dt][p][fm][d] == W2[fm*P + p, dt*P + d]
    w2t = np.ascontiguousarray(
        W2[e].reshape(FM, P, NDT, P).transpose(2, 1, 0, 3)
    ).astype(wdt)
    b1t = np.ascontiguousarray(b1[e].reshape(FM, P).T)   # [P, FM]
    b2t = np.ascontiguousarray(b2[e].reshape(NDT, P).T)  # [P, NDT]
    return {"xt": xt, "w1t": w1t, "w2t": w2t, "b1t": b1t, "b2t": b2t}


def get_nc(mm_dt=None, repeat=1):
    if mm_dt is None:
        mm_dt = MM_DT
    key = (mm_dt, repeat, LDW_OPT)
    if key not in _cache:
        _cache[key] = _build(mm_dt, repeat)
    return _cache[key]


def make_in_maps(inputs, W1, b1, W2, b2, mm_dt=None):
    inputs = np.asarray(inputs, dtype=np.float32)
    W1 = np.asarray(W1, dtype=np.float32)
    b1 = np.asarray(b1, dtype=np.float32)
    W2 = np.asarray(W2, dtype=np.float32)
    b2 = np.asarray(b2, dtype=np.float32)
    wdt = _wire_np_dtype(mm_dt if mm_dt is not None else MM_DT)
    return [_prep_core_inputs(inputs, W1, b1, W2, b2, e, wdt) for e in range(E)]


def assemble_output(per_core_y):
    """per_core_y: list of E arrays [D, CAP] (yT layout) -> [1, E*CAP, D]."""
    out = np.empty((1, E * CAP, D), dtype=np.float32)
    for e in range(E):
        out[0, e * CAP:(e + 1) * CAP, :] = per_core_y[e].T
    return out


def kernel(inputs, W1, b1, W2, b2):
    nc = get_nc()
    in_maps = make_in_maps(inputs, W1, b1, W2, b2)
    # The axon-tunneled devices occasionally come up wedged from a previous
    # process (NRT_EXEC_UNIT_UNRECOVERABLE); a backend reset + retry recovers.
    last_err = None
    for attempt in range(3):
        try:
            res = run_bass_kernel_spmd(nc, in_maps, list(range(E))).results
            break
        except Exception as err:  # noqa: BLE001
            last_err = err
            import time as _time

            try:
                import jax as _jax
                import jax.extend.backend as _jxb

                _jax.clear_caches()
                _jxb.clear_backends()
            except Exception:  # noqa: BLE001
                pass
            _time.sleep(10.0 * (attempt + 1))
    else:
        raise last_err
    return assemble_output([res[e]["y"] for e in range(E)])


if __name__ == "__main__":
    rng = np.random.default_rng(0)
    ins = {
        "inputs": rng.standard_normal((1, E * CAP, D), dtype=np.float32),
        "W1": rng.standard_normal((E, D, F), dtype=np.float32) / np.sqrt(D),
        "b1": np.zeros((E, F), np.float32),
        "W2": rng.standard_normal((E, F, D), dtype=np.float32) / np.sqrt(F),
        "b2": np.zeros((E, D), np.float32),
    }
    y = kernel(**ins)
    print("out", y.shape, y.dtype, float(np.abs(y).mean()))
